# revision 5
# baseline (speedup 1.0000x reference)
"""BatchNeuralKB kernel for Trainium2 (Bass/Tile), 8-core data-parallel.

Per example b: scores = exp(-||q_b - f_{b,j}||^2) over facts j < nb_facts[b],
output = max_j scores (0 when masked out). q/f are concatenated
[rel, arg1, arg2] embeddings of dim 3*256 = 768.

Sharding: batch dim 128 -> 16 examples per core, no cross-core comms.

Host-side marshalling (layout + dtype cast only, no reductions): the three
fact tensors are concatenated to fact_cat [*, F, 768] (optionally cast to
bf16 -> halves the HBM stream); q is concatenated and replicated across the
128 partitions; the nb_facts mask threshold is cast to f32 and replicated.

Per core pipeline (16 examples x 16 fact-chunks of 128 facts) uses
||q - f||^2 = ||f||^2 - 2 q.f + ||q||^2 so the compute engines are fully
decoupled (each reads only the raw fact tile):
  steady loop (per [128, 768] fact slice):
    - DMA: one dma_start per EXAMPLE ([128, 16*768], dma="big") instead of
      per chunk -- dma_start costs the issuing sequencer ~0.6us, so 256
      small DMAs serialize at ~145us on SP; 16 big ones cost ~10us.
    - DVE: scalar_tensor_tensor (f * -2) * q_bcast, sum -> -2 q.f  [1 pass]
    - norm pass sum(f^2): split between ACT (activation Square + accum) and
      DVE ((f*1)*f + accum) by the dve_k knob (of every 8 slices, dve_k go
      to DVE) to balance engine occupancy.
  tail (once): m = sum(f^2) + (-2 q.f); scores = exp(-m - qq) via the Exp
  bias; mask (idx < nb) + multiply + per-example max over chunks on DVE ->
  allmax [128, 16]; the final 128-way max happens in the host-side gather
  (8KB/core).

Numerics: with bf16 facts the squared distances (~1300 +- 70 on this data
regime) carry an O(1) absolute error; scores = exp(-d^2) are identical
because exp underflows f32 at d^2 > ~103 and the kernel's masked-max only
needs d^2 accuracy where scores are representable.

Modes "dma" / "comp" are measurement-only ablations (DMA stream only /
compute from a resident tile only).
"""

import numpy as np
from contextlib import ExitStack

import ml_dtypes

import concourse.bass as bass
import concourse.bacc as bacc
import concourse.tile as tile
from concourse import mybir
from concourse.bass_utils import run_bass_kernel_spmd

B, F, E = 128, 2048, 256
D3 = 3 * E  # 768
N_CORES = 8
BPC = B // N_CORES  # 16 examples per core
CHUNK = 128  # facts per tile (partition dim)
NCH = F // CHUNK  # 16 chunks per example
NCOL = BPC * NCH  # 256 sq_dist columns per core

_f32 = mybir.dt.float32

_DT = {"f32": (_f32, np.float32), "bf16": (mybir.dt.bfloat16, ml_dtypes.bfloat16)}

# Default (graded) configuration; test.py/bench.py may override.
CFG = dict(mode="expan", dtype="bf16", dma="big", dve_k=5,
           dma_engines=("sync",), facts_bufs=4, scr_bufs=8)

_cache = {}


def _build_program(mode="expan", dtype="f32", dma="small", dve_k=0, pool_k=0,
                   dma_engines=("sync",), facts_bufs=20, scr_bufs=8, repeat=1):
    fdt, _ = _DT[dtype]
    nc = bacc.Bacc("TRN2", target_bir_lowering=False, debug=False)

    q_in = nc.dram_tensor("q_rep", [CHUNK, BPC * D3], fdt, kind="ExternalInput")
    fcat = nc.dram_tensor("fact_cat", [BPC, F, D3], fdt, kind="ExternalInput")
    nb_in = nc.dram_tensor("nb_rep", [CHUNK, NCOL], _f32, kind="ExternalInput")
    out_t = nc.dram_tensor("out", [CHUNK, BPC], _f32, kind="ExternalOutput")

    # Constant fact-index tile: idx[p, b*NCH + j] = j*CHUNK + p (fp32-exact)
    idx_np = np.tile(
        (np.arange(NCH)[None, :] * CHUNK + np.arange(CHUNK)[:, None]).astype(
            np.float32
        ),
        (1, BPC),
    )
    idx_const = nc.inline_tensor(idx_np, name="idx_const")

    Sq = mybir.ActivationFunctionType.Square
    Ex = mybir.ActivationFunctionType.Exp

    with tile.TileContext(nc) as tc, ExitStack() as ctx:
        facts = ctx.enter_context(tc.tile_pool(name="facts", bufs=facts_bufs))
        scr = ctx.enter_context(tc.tile_pool(name="scr", bufs=scr_bufs))
        small = ctx.enter_context(tc.tile_pool(name="small", bufs=1))

        idx_sb = small.tile([CHUNK, NCOL], _f32, tag="idx")
        nc.sync.dma_start(idx_sb[:], idx_const.ap()[:, :])
        nb_rep = small.tile([CHUNK, NCOL], _f32, tag="nbr")
        nc.sync.dma_start(nb_rep[:], nb_in.ap()[:, :])
        q_all = small.tile([CHUNK, BPC * D3], fdt, tag="qall")
        nc.sync.dma_start(q_all[:], q_in.ap()[:, :])

        # Touch preloaded tiles once per consuming engine so steady-state
        # consumers carry at most one new semaphore wait each.
        obs = small.tile([CHUNK, 4], _f32, tag="obs")
        nc.vector.tensor_copy(obs[:, 0:1], idx_sb[:, 0:1])
        nc.vector.tensor_copy(obs[:, 1:2], nb_rep[:, 0:1])

        m_all = small.tile([CHUNK, NCOL], _f32, tag="mall")
        engs = [getattr(nc, e) for e in dma_engines]

        # qq[b] = sum(q_b^2), negated for use as the Exp bias later.
        qf_all = small.tile([CHUNK, NCOL], _f32, tag="qfall")
        qq16 = small.tile([CHUNK, BPC], _f32, tag="qq16")
        for b in range(BPC):
            q_scr = scr.tile([CHUNK, D3], fdt, tag="act_scr")
            nc.scalar.activation(
                q_scr[:],
                q_all[:, b * D3 : (b + 1) * D3],
                Sq,
                accum_out=qq16[:, b : b + 1],
            )
        qqn16 = small.tile([CHUNK, BPC], _f32, tag="qqn16")
        nc.vector.tensor_scalar_mul(qqn16[:], qq16[:], -1.0)

        if mode == "comp":
            ft0 = small.tile([CHUNK, D3], fdt, tag="ft0")
            nc.sync.dma_start(ft0[:], fcat.ap()[0, 0:CHUNK, :])

        for _rep in range(repeat):
          for b in range(BPC):
              q_bc = q_all[:, b * D3 : (b + 1) * D3]
              ftb = None
              if dma == "big" and mode != "comp":
                  ftb = facts.tile([CHUNK, NCH * D3], fdt, tag="ftb")
                  engs[b % len(engs)].dma_start(
                      ftb[:].rearrange("p (j d) -> p j d", j=NCH),
                      fcat.ap()[b].rearrange("(j p) d -> p j d", p=CHUNK),
                  )
              for j in range(NCH):
                  col = b * NCH + j
                  if mode == "comp":
                      sl = ft0[:]
                  elif dma == "big":
                      sl = ftb[:, j * D3 : (j + 1) * D3]
                  else:
                      ft = facts.tile([CHUNK, D3], fdt, tag="ft")
                      r0 = slice(j * CHUNK, (j + 1) * CHUNK)
                      engs[col % len(engs)].dma_start(ft[:], fcat.ap()[b, r0, :])
                      sl = ft[:]
                  if mode == "dma":
                      continue

                  # Decoupled: DVE computes -2*q.f; norm pass on ACT or DVE.
                  d_scr = scr.tile([CHUNK, D3], fdt, tag="dve_scr")
                  nc.vector.scalar_tensor_tensor(
                      out=d_scr[:],
                      in0=sl,
                      scalar=-2.0,
                      in1=q_bc,
                      op0=mybir.AluOpType.mult,
                      op1=mybir.AluOpType.mult,
                      accum_out=qf_all[:, col : col + 1],
                  )
                  if (col % 8) < dve_k:
                      d_scr2 = scr.tile([CHUNK, D3], fdt, tag="dve_scr")
                      nc.vector.scalar_tensor_tensor(
                          out=d_scr2[:],
                          in0=sl,
                          scalar=1.0,
                          in1=sl,
                          op0=mybir.AluOpType.mult,
                          op1=mybir.AluOpType.mult,
                          accum_out=m_all[:, col : col + 1],
                      )
                  elif (col % 8) < dve_k + pool_k:
                      p_scr = scr.tile([CHUNK, D3], fdt, tag="pool_scr")
                      nc.gpsimd.scalar_tensor_tensor(
                          out=p_scr[:],
                          in0=sl,
                          scalar=1.0,
                          in1=sl,
                          op0=mybir.AluOpType.mult,
                          op1=mybir.AluOpType.mult,
                          accum_out=m_all[:, col : col + 1],
                      )
                  else:
                      a_scr = scr.tile([CHUNK, D3], fdt, tag="act_scr")
                      nc.scalar.activation(
                          a_scr[:], sl, Sq, accum_out=m_all[:, col : col + 1]
                      )

        if mode == "dma":
            nc.vector.tensor_copy(m_all[:, 0:NCOL], idx_sb[:, 0:NCOL])
            nc.vector.tensor_copy(qf_all[:, 0:NCOL], idx_sb[:, 0:NCOL])

        # Tail: scores = exp(-sq_dist), mask, per-example max over chunks.
        sc = small.tile([CHUNK, NCOL], _f32, tag="sc")
        nc.vector.tensor_add(m_all[:], m_all[:], qf_all[:])
        for b in range(BPC):
            bsl = slice(b * NCH, (b + 1) * NCH)
            nc.scalar.activation(
                sc[:, bsl],
                m_all[:, bsl],
                Ex,
                scale=-1.0,
                bias=qqn16[:, b : b + 1],
            )
        mask = small.tile([CHUNK, NCOL], _f32, tag="mask")
        nc.vector.tensor_tensor(
            mask[:], idx_sb[:], nb_rep[:], op=mybir.AluOpType.is_lt
        )
        msc = small.tile([CHUNK, NCOL], _f32, tag="msc")
        nc.vector.tensor_mul(msc[:], sc[:], mask[:])
        allmax = small.tile([CHUNK, BPC], _f32, tag="allmax")
        nc.vector.tensor_reduce(
            allmax[:],
            msc[:].rearrange("p (b j) -> p b j", b=BPC),
            axis=mybir.AxisListType.X,
            op=mybir.AluOpType.max,
        )
        nc.sync.dma_start(out_t.ap()[:, :], allmax[:])

    nc.compile()
    return nc


def _get_program():
    if "nc" not in _cache:
        _cache["nc"] = _build_program(**CFG)
    return _cache["nc"]


def _make_in_maps(rel, arg1, arg2, fact_rel, fact_arg1, fact_arg2, nb_facts,
                  dtype=None):
    _, ndt = _DT[dtype or CFG["dtype"]]
    q_cat = np.concatenate(
        [
            np.asarray(rel, dtype=np.float32),
            np.asarray(arg1, dtype=np.float32),
            np.asarray(arg2, dtype=np.float32),
        ],
        axis=1,
    ).astype(ndt)  # [B, 768]
    nb_f32 = np.asarray(nb_facts).astype(np.float32)
    fact_cat = np.concatenate(
        [
            np.asarray(fact_rel, dtype=np.float32),
            np.asarray(fact_arg1, dtype=np.float32),
            np.asarray(fact_arg2, dtype=np.float32),
        ],
        axis=2,
    ).astype(ndt)  # [B, F, 768]

    in_maps = []
    for c in range(N_CORES):
        s = slice(c * BPC, (c + 1) * BPC)
        q_flat = q_cat[s].reshape(1, BPC * D3)
        nb_flat = np.repeat(nb_f32[s], NCH).reshape(1, NCOL)
        in_maps.append(
            {
                "q_rep": np.ascontiguousarray(
                    np.broadcast_to(q_flat, (CHUNK, BPC * D3))
                ),
                "fact_cat": fact_cat[s],
                "nb_rep": np.ascontiguousarray(np.broadcast_to(nb_flat, (CHUNK, NCOL))),
            }
        )
    return in_maps


def kernel(rel, arg1, arg2, fact_rel, fact_arg1, fact_arg2, nb_facts):
    nc = _get_program()
    in_maps = _make_in_maps(
        rel, arg1, arg2, fact_rel, fact_arg1, fact_arg2, nb_facts
    )
    res = run_bass_kernel_spmd(nc, in_maps, list(range(N_CORES))).results
    # res[c]["out"]: [128, BPC] per-partition chunk maxima; final 128-way max
    # per example happens here in the gather.
    out = np.concatenate(
        [np.asarray(res[c]["out"]).max(axis=0) for c in range(N_CORES)]
    )
    return out.astype(np.float32)


# revision 21
# speedup vs baseline: 2.3386x; 2.3386x over previous
"""BatchNeuralKB kernel for Trainium2 (Bass/Tile), 8-core data-parallel.

Per example b: scores = exp(-||q_b - f_{b,j}||^2) over facts j < nb_facts[b],
output = max_j scores (0 when masked out). q/f are concatenated
[rel, arg1, arg2] embeddings of dim 3*256 = 768.

Sharding: batch dim 128 -> 16 examples per core, no cross-core comms.
Host-side marshalling is layout + dtype cast only (no reductions): facts are
concatenated and cast to bf16 (halves the HBM stream; d^2 error stays far
inside the exp-underflow margin), queries replicated across partitions, the
nb_facts mask threshold cast to f32.

Why this shape: every per-fact reduction (768 -> 1) on the DVE/ACT engines
runs at 1 elem/lane/cycle (no 2x/4x DVE modes exist for reducing ops), so a
full dot-product pass costs ~236us/core and a Square+accum pass ~268us --
those two passes are the whole problem. The only engine that reduces faster
is the PE array (128 lanes x 2.4GHz as a matmul contraction). So examples
are split per core (mode="hybridpe", nb_ex=13):

  B examples (13): facts arrive TRANSPOSED [dim, fact] (host layout).
    - DVE squares the whole example tile in one 2x-mode bf16 multiply
      (fp16 output: bf16 squares would cost ~0.1 absolute on d^2).
    - PE: per 128x128 block, facts as stationary, moving cols = the
      (-2q) query column (dot) and a ones column over the squared facts
      (norm); PSUM accumulates the 6 dim-blocks; output lands directly as
      [fact-partition, chunk] columns, drained by two tiny DVE copies.
  A examples (3): classic decoupled passes from [fact, dim] tiles --
    DVE scalar_tensor_tensor (f*-2)*q sum -> -2 q.f, ACT Square+accum ->
    sum(f^2) -- keeping DVE/ACT busy while PE works the B examples.

  DMA: one dma_start per EXAMPLE ([128, 16*768]) -- dma_start costs the
  issuing sequencer ~0.6us, so 256 per-chunk DMAs would serialize at
  ~145us on SP; 16 big ones cost ~10us. Stream is bf16: 48MB/core at the
  measured ~345GB/s/core = ~139us, overlapped with compute.

  tail (once): m = sum(f^2) + (-2 q.f); scores = exp(-m - qq) via the Exp
  bias; mask (idx < nb) + multiply + per-example max over chunks on DVE ->
  allmax [128, 16]; the final 128-way max happens in the host-side gather
  (8KB/core).

Measured (hw For_i loop slope, 8 cores): hybridpe ~207us/exec vs ~279us for
the pure DVE/ACT bf16 variant; DMA-only floor ~139us.

Numerics: bf16 facts give d^2 errors O(0.3) absolute; on this data regime
(min d^2 ~ 1190 >> 103) every score underflows f32 to exactly 0 either way;
planted-match selftest shows ~2% score error where scores are representable.

Modes "dma" / "comp" are measurement-only ablations; mode="expan" is the
pure DVE/ACT variant; hw_repeat wraps the streaming loop in a hardware
For_i for wall-clock slope timing.
"""

import numpy as np
from contextlib import ExitStack

import ml_dtypes

import concourse.bass as bass
import concourse.bacc as bacc
import concourse.tile as tile
from concourse import mybir
from concourse.bass_utils import run_bass_kernel_spmd

B, F, E = 128, 2048, 256
D3 = 3 * E  # 768
N_CORES = 8
BPC = B // N_CORES  # 16 examples per core
CHUNK = 128  # facts per tile (partition dim)
NCH = F // CHUNK  # 16 chunks per example
NCOL = BPC * NCH  # 256 sq_dist columns per core

_f32 = mybir.dt.float32

_DT = {"f32": (_f32, np.float32), "bf16": (mybir.dt.bfloat16, ml_dtypes.bfloat16)}

# Default (graded) configuration; test.py/bench.py may override.
CFG = dict(mode="hybridpe", dtype="bf16", dma="big", nb_ex=13,
           dma_engines=("sync",), facts_bufs=2, scr_bufs=8)

_cache = {}


def _build_program(mode="expan", dtype="f32", dma="small", dve_k=0, pool_k=0,
                   dot_pool_k=0, dve_k16=0, nb_ex=0, dma_engines=("sync",),
                   facts_bufs=20, scr_bufs=8, repeat=1, hw_repeat=1):
    fdt, _ = _DT[dtype]
    NB = nb_ex if mode == "hybridpe" else 0
    NA = BPC - NB
    NDB = D3 // CHUNK  # 6 dim-blocks of 128 for the transposed PE path
    nc = bacc.Bacc("TRN2", target_bir_lowering=False, debug=False)

    q_in = nc.dram_tensor("q_rep", [CHUNK, BPC * D3], fdt, kind="ExternalInput")
    fcat = nc.dram_tensor("fact_cat", [NA, F, D3], fdt, kind="ExternalInput")
    if NB:
        # B examples, dim-major: fact_t[b, db, d, f] = fact[b, f, db*128+d]
        fact_t = nc.dram_tensor(
            "fact_t", [NB, NDB, CHUNK, F], fdt, kind="ExternalInput"
        )
        # q_t[d, b*NDB+db] = -2 * q[b, db*128+d]
        qt_in = nc.dram_tensor("q_t", [CHUNK, NB * NDB], fdt, kind="ExternalInput")
    nb_in = nc.dram_tensor("nb_rep", [CHUNK, NCOL], _f32, kind="ExternalInput")
    out_t = nc.dram_tensor("out", [CHUNK, BPC], _f32, kind="ExternalOutput")

    # Constant fact-index tile: idx[p, b*NCH + j] = j*CHUNK + p (fp32-exact)
    idx_np = np.tile(
        (np.arange(NCH)[None, :] * CHUNK + np.arange(CHUNK)[:, None]).astype(
            np.float32
        ),
        (1, BPC),
    )
    idx_const = nc.inline_tensor(idx_np, name="idx_const")

    Sq = mybir.ActivationFunctionType.Square
    Ex = mybir.ActivationFunctionType.Exp

    with tile.TileContext(nc) as tc, ExitStack() as ctx:
        facts = ctx.enter_context(tc.tile_pool(name="facts", bufs=facts_bufs))
        scr = ctx.enter_context(tc.tile_pool(name="scr", bufs=scr_bufs))
        small = ctx.enter_context(tc.tile_pool(name="small", bufs=1))
        if NB:
            bfacts = ctx.enter_context(tc.tile_pool(name="bfacts", bufs=2))
            sqp = ctx.enter_context(tc.tile_pool(name="sqp", bufs=2))
            psum = ctx.enter_context(
                tc.tile_pool(name="psum", bufs=4, space="PSUM")
            )
            # Squares go to fp16 (not bf16): the 8-bit bf16 mantissa on f^2
            # costs ~0.1 absolute on d^2; fp16's 11 bits keep it ~0.01.
            _sq_dt = mybir.dt.float16
            ones_np = np.ones((CHUNK, 1), dtype=np.float16)
            ones_const = nc.inline_tensor(ones_np, name="ones_const")
            ones_sb = small.tile([CHUNK, 1], _sq_dt, tag="ones")
            nc.sync.dma_start(ones_sb[:], ones_const.ap()[:, :])
            qt_sb = small.tile([CHUNK, NB * NDB], fdt, tag="qt")
            nc.sync.dma_start(qt_sb[:], qt_in.ap()[:, :])

        idx_sb = small.tile([CHUNK, NCOL], _f32, tag="idx")
        nc.sync.dma_start(idx_sb[:], idx_const.ap()[:, :])
        nb_rep = small.tile([CHUNK, NCOL], _f32, tag="nbr")
        nc.sync.dma_start(nb_rep[:], nb_in.ap()[:, :])
        q_all = small.tile([CHUNK, BPC * D3], fdt, tag="qall")
        nc.sync.dma_start(q_all[:], q_in.ap()[:, :])

        # Touch preloaded tiles once per consuming engine so steady-state
        # consumers carry at most one new semaphore wait each.
        obs = small.tile([CHUNK, 4], _f32, tag="obs")
        nc.vector.tensor_copy(obs[:, 0:1], idx_sb[:, 0:1])
        nc.vector.tensor_copy(obs[:, 1:2], nb_rep[:, 0:1])

        m_all = small.tile([CHUNK, NCOL], _f32, tag="mall")
        engs = [getattr(nc, e) for e in dma_engines]

        # qq[b] = sum(q_b^2), negated for use as the Exp bias later.
        qf_all = small.tile([CHUNK, NCOL], _f32, tag="qfall")
        qq16 = small.tile([CHUNK, BPC], _f32, tag="qq16")
        for b in range(BPC):
            q_scr = scr.tile([CHUNK, D3], fdt, tag="act_scr")
            nc.scalar.activation(
                q_scr[:],
                q_all[:, b * D3 : (b + 1) * D3],
                Sq,
                accum_out=qq16[:, b : b + 1],
            )
        qqn16 = small.tile([CHUNK, BPC], _f32, tag="qqn16")
        nc.vector.tensor_scalar_mul(qqn16[:], qq16[:], -1.0)

        if mode == "comp":
            ft0 = small.tile([CHUNK, D3], fdt, tag="ft0")
            nc.sync.dma_start(ft0[:], fcat.ap()[0, 0:CHUNK, :])

        loop_cm = tc.For_i(0, hw_repeat) if hw_repeat > 1 else None
        if loop_cm is not None:
            loop_cm.__enter__()
        for _rep in range(repeat):
          for bi in range(NB):
              # PE path: facts stationary [128 dims, 128 facts]; moving cols
              # are the (-2q) query column (dot) and ones (norm over squared
              # facts from one 2x-mode DVE multiply). PSUM accumulates the 6
              # dim-blocks; output lands as [fact-partition, chunk] columns.
              bg = NA + bi
              ftb_t = bfacts.tile([CHUNK, NDB * F], fdt, tag="ftbt")
              nc.sync.dma_start(
                  ftb_t[:].rearrange("p (db f) -> p db f", db=NDB),
                  fact_t.ap()[bi].rearrange("db p f -> p db f"),
              )
              sq = sqp.tile([CHUNK, NDB * F], mybir.dt.float16, tag="sq")
              nc.vector.tensor_mul(sq[:], ftb_t[:], ftb_t[:])
              pd = psum.tile([CHUNK, NCH], _f32, tag="pd")
              pn = psum.tile([CHUNK, NCH], _f32, tag="pn")
              for fb in range(NCH):
                  for db in range(NDB):
                      blk = slice(db * F + fb * CHUNK, db * F + (fb + 1) * CHUNK)
                      nc.tensor.matmul(
                          pd[:, fb : fb + 1],
                          ftb_t[:, blk],
                          qt_sb[:, bi * NDB + db : bi * NDB + db + 1],
                          start=(db == 0),
                          stop=(db == NDB - 1),
                      )
                  for db in range(NDB):
                      blk = slice(db * F + fb * CHUNK, db * F + (fb + 1) * CHUNK)
                      nc.tensor.matmul(
                          pn[:, fb : fb + 1],
                          sq[:, blk],
                          ones_sb[:],
                          start=(db == 0),
                          stop=(db == NDB - 1),
                      )
              cols = slice(bg * NCH, (bg + 1) * NCH)
              nc.vector.tensor_copy(qf_all[:, cols], pd[:])
              nc.vector.tensor_copy(m_all[:, cols], pn[:])

          for b in range(NA):
              q_bc = q_all[:, b * D3 : (b + 1) * D3]
              ftb = None
              if dma == "big" and mode != "comp":
                  ftb = facts.tile([CHUNK, NCH * D3], fdt, tag="ftb")
                  engs[b % len(engs)].dma_start(
                      ftb[:].rearrange("p (j d) -> p j d", j=NCH),
                      fcat.ap()[b].rearrange("(j p) d -> p j d", p=CHUNK),
                  )
              for j in range(NCH):
                  col = b * NCH + j
                  if mode == "comp":
                      sl = ft0[:]
                  elif dma == "big":
                      sl = ftb[:, j * D3 : (j + 1) * D3]
                  else:
                      ft = facts.tile([CHUNK, D3], fdt, tag="ft")
                      r0 = slice(j * CHUNK, (j + 1) * CHUNK)
                      engs[col % len(engs)].dma_start(ft[:], fcat.ap()[b, r0, :])
                      sl = ft[:]
                  if mode == "dma":
                      continue

                  # Decoupled passes; dot -2*q.f on DVE (or Pool for a
                  # dot_pool_k slice share), norm sum(f^2) on ACT/DVE/Pool.
                  dot_eng = (
                      nc.gpsimd if (col % 16) < dot_pool_k else nc.vector
                  )
                  d_scr = scr.tile([CHUNK, D3], fdt, tag="dve_scr")
                  dot_eng.scalar_tensor_tensor(
                      out=d_scr[:],
                      in0=sl,
                      scalar=-2.0,
                      in1=q_bc,
                      op0=mybir.AluOpType.mult,
                      op1=mybir.AluOpType.mult,
                      accum_out=qf_all[:, col : col + 1],
                  )
                  if (col % 8) < dve_k or (col % 16) < dve_k16:
                      d_scr2 = scr.tile([CHUNK, D3], fdt, tag="dve_scr")
                      nc.vector.scalar_tensor_tensor(
                          out=d_scr2[:],
                          in0=sl,
                          scalar=1.0,
                          in1=sl,
                          op0=mybir.AluOpType.mult,
                          op1=mybir.AluOpType.mult,
                          accum_out=m_all[:, col : col + 1],
                      )
                  elif (col % 8) < dve_k + pool_k:
                      p_scr = scr.tile([CHUNK, D3], fdt, tag="pool_scr")
                      nc.gpsimd.scalar_tensor_tensor(
                          out=p_scr[:],
                          in0=sl,
                          scalar=1.0,
                          in1=sl,
                          op0=mybir.AluOpType.mult,
                          op1=mybir.AluOpType.mult,
                          accum_out=m_all[:, col : col + 1],
                      )
                  else:
                      a_scr = scr.tile([CHUNK, D3], fdt, tag="act_scr")
                      nc.scalar.activation(
                          a_scr[:], sl, Sq, accum_out=m_all[:, col : col + 1]
                      )

        if loop_cm is not None:
            loop_cm.__exit__(None, None, None)

        if mode == "dma":
            nc.vector.tensor_copy(m_all[:, 0:NCOL], idx_sb[:, 0:NCOL])
            nc.vector.tensor_copy(qf_all[:, 0:NCOL], idx_sb[:, 0:NCOL])

        # Tail: scores = exp(-sq_dist), mask, per-example max over chunks.
        sc = small.tile([CHUNK, NCOL], _f32, tag="sc")
        nc.vector.tensor_add(m_all[:], m_all[:], qf_all[:])
        for b in range(BPC):
            bsl = slice(b * NCH, (b + 1) * NCH)
            nc.scalar.activation(
                sc[:, bsl],
                m_all[:, bsl],
                Ex,
                scale=-1.0,
                bias=qqn16[:, b : b + 1],
            )
        mask = small.tile([CHUNK, NCOL], _f32, tag="mask")
        nc.vector.tensor_tensor(
            mask[:], idx_sb[:], nb_rep[:], op=mybir.AluOpType.is_lt
        )
        msc = small.tile([CHUNK, NCOL], _f32, tag="msc")
        nc.vector.tensor_mul(msc[:], sc[:], mask[:])
        allmax = small.tile([CHUNK, BPC], _f32, tag="allmax")
        nc.vector.tensor_reduce(
            allmax[:],
            msc[:].rearrange("p (b j) -> p b j", b=BPC),
            axis=mybir.AxisListType.X,
            op=mybir.AluOpType.max,
        )
        nc.sync.dma_start(out_t.ap()[:, :], allmax[:])

    nc.compile()
    return nc


def _get_program():
    if "nc" not in _cache:
        _cache["nc"] = _build_program(**CFG)
    return _cache["nc"]


def _make_in_maps(rel, arg1, arg2, fact_rel, fact_arg1, fact_arg2, nb_facts,
                  dtype=None, nb_ex=None):
    _, ndt = _DT[dtype or CFG["dtype"]]
    if nb_ex is None:
        nb_ex = CFG.get("nb_ex", 0) if CFG.get("mode") == "hybridpe" else 0
    NB, NA = nb_ex, BPC - nb_ex
    NDB = D3 // CHUNK
    q_f32 = np.concatenate(
        [
            np.asarray(rel, dtype=np.float32),
            np.asarray(arg1, dtype=np.float32),
            np.asarray(arg2, dtype=np.float32),
        ],
        axis=1,
    )  # [B, 768]
    q_cat = q_f32.astype(ndt)
    nb_f32 = np.asarray(nb_facts).astype(np.float32)
    fact_cat = np.concatenate(
        [
            np.asarray(fact_rel, dtype=np.float32),
            np.asarray(fact_arg1, dtype=np.float32),
            np.asarray(fact_arg2, dtype=np.float32),
        ],
        axis=2,
    ).astype(ndt)  # [B, F, 768]

    in_maps = []
    for c in range(N_CORES):
        s = slice(c * BPC, (c + 1) * BPC)
        q_flat = q_cat[s].reshape(1, BPC * D3)
        nb_flat = np.repeat(nb_f32[s], NCH).reshape(1, NCOL)
        m = {
            "q_rep": np.ascontiguousarray(
                np.broadcast_to(q_flat, (CHUNK, BPC * D3))
            ),
            "fact_cat": fact_cat[s][:NA],
            "nb_rep": np.ascontiguousarray(np.broadcast_to(nb_flat, (CHUNK, NCOL))),
        }
        if NB:
            fb = fact_cat[s][NA:]  # [NB, F, 768]
            m["fact_t"] = np.ascontiguousarray(
                fb.transpose(0, 2, 1).reshape(NB, NDB, CHUNK, F)
            )
            qb = (-2.0 * q_f32[s][NA:]).astype(ndt)  # [NB, 768]
            m["q_t"] = np.ascontiguousarray(
                qb.reshape(NB, NDB, CHUNK).transpose(2, 0, 1).reshape(CHUNK, NB * NDB)
            )
        in_maps.append(m)
    return in_maps


def kernel(rel, arg1, arg2, fact_rel, fact_arg1, fact_arg2, nb_facts):
    nc = _get_program()
    in_maps = _make_in_maps(
        rel, arg1, arg2, fact_rel, fact_arg1, fact_arg2, nb_facts
    )
    res = run_bass_kernel_spmd(nc, in_maps, list(range(N_CORES))).results
    # res[c]["out"]: [128, BPC] per-partition chunk maxima; final 128-way max
    # per example happens here in the gather.
    out = np.concatenate(
        [np.asarray(res[c]["out"]).max(axis=0) for c in range(N_CORES)]
    )
    return out.astype(np.float32)


# revision 25
# speedup vs baseline: 2.5011x; 1.0695x over previous
"""BatchNeuralKB kernel for Trainium2 (Bass/Tile), 8-core data-parallel.

Per example b: scores = exp(-||q_b - f_{b,j}||^2) over facts j < nb_facts[b],
output = max_j scores (0 when masked out). q/f are concatenated
[rel, arg1, arg2] embeddings of dim 3*256 = 768.

Sharding: batch dim 128 -> 16 examples per core, no cross-core comms.
Host-side marshalling is layout + dtype cast only (no reductions): facts are
concatenated and cast to bf16 (halves the HBM stream; d^2 error stays far
inside the exp-underflow margin), queries replicated across partitions, the
nb_facts mask threshold cast to f32.

Why this shape: every per-fact reduction (768 -> 1) on the DVE/ACT engines
runs at 1 elem/lane/cycle (no 2x/4x DVE modes exist for reducing ops), so a
full dot-product pass costs ~236us/core and a Square+accum pass ~268us --
those two passes are the whole problem. The only engine that reduces faster
is the PE array (128 lanes x 2.4GHz as a matmul contraction). So examples
are split per core (mode="hybridpe", nb_ex=13):

  B examples (13): facts arrive TRANSPOSED [dim, fact] (host layout).
    - DVE squares the whole example tile in one 2x-mode bf16 multiply
      (fp16 output: bf16 squares would cost ~0.1 absolute on d^2).
    - PE: per 128x128 block, facts as stationary, moving cols = the
      (-2q) query column (dot) and a ones column over the squared facts
      (norm); PSUM accumulates the 6 dim-blocks; output lands directly as
      [fact-partition, chunk] columns, drained by two tiny DVE copies.
  A examples (3): classic decoupled passes from [fact, dim] tiles --
    DVE scalar_tensor_tensor (f*-2)*q sum -> -2 q.f, ACT Square+accum ->
    sum(f^2) -- keeping DVE/ACT busy while PE works the B examples.

  DMA: one dma_start per EXAMPLE ([128, 16*768]) -- dma_start costs the
  issuing sequencer ~0.6us, so 256 per-chunk DMAs would serialize at
  ~145us on SP; 16 big ones cost ~10us. Stream is bf16: 48MB/core at the
  measured ~345GB/s/core = ~139us, overlapped with compute.

  tail (once): m = sum(f^2) + (-2 q.f); scores = exp(-m - qq) via the Exp
  bias; mask (idx < nb) + multiply + per-example max over chunks on DVE ->
  allmax [128, 16]; the final 128-way max happens in the host-side gather
  (8KB/core).

Measured (hw For_i loop slope, 8 cores): hybridpe ~207us/exec vs ~279us for
the pure DVE/ACT bf16 variant; DMA-only floor ~139us.

Numerics: bf16 facts give d^2 errors O(0.3) absolute; on this data regime
(min d^2 ~ 1190 >> 103) every score underflows f32 to exactly 0 either way;
planted-match selftest shows ~2% score error where scores are representable.

Modes "dma" / "comp" are measurement-only ablations; mode="expan" is the
pure DVE/ACT variant; hw_repeat wraps the streaming loop in a hardware
For_i for wall-clock slope timing.
"""

import numpy as np
from contextlib import ExitStack

import ml_dtypes

import concourse.bass as bass
import concourse.bacc as bacc
import concourse.tile as tile
from concourse import mybir
from concourse.bass_utils import run_bass_kernel_spmd

B, F, E = 128, 2048, 256
D3 = 3 * E  # 768
N_CORES = 8
BPC = B // N_CORES  # 16 examples per core
CHUNK = 128  # facts per tile (partition dim)
NCH = F // CHUNK  # 16 chunks per example
NCOL = BPC * NCH  # 256 sq_dist columns per core

_f32 = mybir.dt.float32

_DT = {"f32": (_f32, np.float32), "bf16": (mybir.dt.bfloat16, ml_dtypes.bfloat16)}

# Default (graded) configuration; test.py/bench.py may override.
CFG = dict(mode="hybridpe", dtype="bf16", dma="big", nb_ex=13, sq_act_k=2,
           dma_engines=("sync",), facts_bufs=1, bfacts_bufs=3, scr_bufs=8)

_cache = {}


def _build_program(mode="expan", dtype="f32", dma="small", dve_k=0, pool_k=0,
                   dot_pool_k=0, dve_k16=0, nb_ex=0, sq_act_k=0,
                   dma_engines=("sync",), facts_bufs=20, scr_bufs=8, repeat=1,
                   hw_repeat=1, bfacts_bufs=2):
    fdt, _ = _DT[dtype]
    NB = nb_ex if mode == "hybridpe" else 0
    NA = BPC - NB
    NDB = D3 // CHUNK  # 6 dim-blocks of 128 for the transposed PE path
    nc = bacc.Bacc("TRN2", target_bir_lowering=False, debug=False)

    q_in = nc.dram_tensor("q_rep", [CHUNK, BPC * D3], fdt, kind="ExternalInput")
    fcat = nc.dram_tensor("fact_cat", [NA, F, D3], fdt, kind="ExternalInput")
    if NB:
        # B examples, dim-major: fact_t[b, db, d, f] = fact[b, f, db*128+d]
        fact_t = nc.dram_tensor(
            "fact_t", [NB, NDB, CHUNK, F], fdt, kind="ExternalInput"
        )
        # q_t[d, b*NDB+db] = -2 * q[b, db*128+d]
        qt_in = nc.dram_tensor("q_t", [CHUNK, NB * NDB], fdt, kind="ExternalInput")
    nb_in = nc.dram_tensor("nb_rep", [CHUNK, NCOL], _f32, kind="ExternalInput")
    out_t = nc.dram_tensor("out", [CHUNK, BPC], _f32, kind="ExternalOutput")

    # Constant fact-index tile: idx[p, b*NCH + j] = j*CHUNK + p (fp32-exact)
    idx_np = np.tile(
        (np.arange(NCH)[None, :] * CHUNK + np.arange(CHUNK)[:, None]).astype(
            np.float32
        ),
        (1, BPC),
    )
    idx_const = nc.inline_tensor(idx_np, name="idx_const")

    Sq = mybir.ActivationFunctionType.Square
    Ex = mybir.ActivationFunctionType.Exp

    with tile.TileContext(nc) as tc, ExitStack() as ctx:
        facts = ctx.enter_context(tc.tile_pool(name="facts", bufs=facts_bufs))
        scr = ctx.enter_context(tc.tile_pool(name="scr", bufs=scr_bufs))
        small = ctx.enter_context(tc.tile_pool(name="small", bufs=1))
        if NB:
            bfacts = ctx.enter_context(tc.tile_pool(name="bfacts", bufs=bfacts_bufs))
            sqp = ctx.enter_context(tc.tile_pool(name="sqp", bufs=2))
            psum = ctx.enter_context(
                tc.tile_pool(name="psum", bufs=4, space="PSUM")
            )
            # Squares go to fp16 (not bf16): the 8-bit bf16 mantissa on f^2
            # costs ~0.1 absolute on d^2; fp16's 11 bits keep it ~0.01.
            _sq_dt = mybir.dt.float16
            ones_np = np.ones((CHUNK, 1), dtype=np.float16)
            ones_const = nc.inline_tensor(ones_np, name="ones_const")
            ones_sb = small.tile([CHUNK, 1], _sq_dt, tag="ones")
            nc.sync.dma_start(ones_sb[:], ones_const.ap()[:, :])
            qt_sb = small.tile([CHUNK, NB * NDB], fdt, tag="qt")
            nc.sync.dma_start(qt_sb[:], qt_in.ap()[:, :])

        idx_sb = small.tile([CHUNK, NCOL], _f32, tag="idx")
        nc.sync.dma_start(idx_sb[:], idx_const.ap()[:, :])
        nb_rep = small.tile([CHUNK, NCOL], _f32, tag="nbr")
        nc.sync.dma_start(nb_rep[:], nb_in.ap()[:, :])
        q_all = small.tile([CHUNK, BPC * D3], fdt, tag="qall")
        nc.sync.dma_start(q_all[:], q_in.ap()[:, :])

        # Touch preloaded tiles once per consuming engine so steady-state
        # consumers carry at most one new semaphore wait each.
        obs = small.tile([CHUNK, 4], _f32, tag="obs")
        nc.vector.tensor_copy(obs[:, 0:1], idx_sb[:, 0:1])
        nc.vector.tensor_copy(obs[:, 1:2], nb_rep[:, 0:1])

        m_all = small.tile([CHUNK, NCOL], _f32, tag="mall")
        engs = [getattr(nc, e) for e in dma_engines]

        # qq[b] = sum(q_b^2), negated for use as the Exp bias later.
        qf_all = small.tile([CHUNK, NCOL], _f32, tag="qfall")
        qq16 = small.tile([CHUNK, BPC], _f32, tag="qq16")
        for b in range(BPC):
            q_scr = scr.tile([CHUNK, D3], fdt, tag="act_scr")
            nc.scalar.activation(
                q_scr[:],
                q_all[:, b * D3 : (b + 1) * D3],
                Sq,
                accum_out=qq16[:, b : b + 1],
            )
        qqn16 = small.tile([CHUNK, BPC], _f32, tag="qqn16")
        nc.vector.tensor_scalar_mul(qqn16[:], qq16[:], -1.0)

        if mode == "comp":
            ft0 = small.tile([CHUNK, D3], fdt, tag="ft0")
            nc.sync.dma_start(ft0[:], fcat.ap()[0, 0:CHUNK, :])

        def emit_B(bi):
            # PE path: facts stationary [128 dims, 128 facts]; moving cols
            # are the (-2q) query column (dot) and ones (norm over squared
            # facts from one 2x-mode DVE multiply). PSUM accumulates the 6
            # dim-blocks; output lands as [fact-partition, chunk] columns.
            bg = NA + bi
            ftb_t = bfacts.tile([CHUNK, NDB * F], fdt, tag="ftbt")
            nc.sync.dma_start(
                ftb_t[:].rearrange("p (db f) -> p db f", db=NDB),
                fact_t.ap()[bi].rearrange("db p f -> p db f"),
            )
            sq = sqp.tile([CHUNK, NDB * F], mybir.dt.float16, tag="sq")
            if (bi % 4) < sq_act_k:
                nc.scalar.activation(sq[:], ftb_t[:], Sq)
            else:
                nc.vector.tensor_mul(sq[:], ftb_t[:], ftb_t[:])
            pd = psum.tile([CHUNK, NCH], _f32, tag="pd")
            pn = psum.tile([CHUNK, NCH], _f32, tag="pn")
            for fb in range(NCH):
                for db in range(NDB):
                    blk = slice(db * F + fb * CHUNK, db * F + (fb + 1) * CHUNK)
                    nc.tensor.matmul(
                        pd[:, fb : fb + 1],
                        ftb_t[:, blk],
                        qt_sb[:, bi * NDB + db : bi * NDB + db + 1],
                        start=(db == 0),
                        stop=(db == NDB - 1),
                    )
                for db in range(NDB):
                    blk = slice(db * F + fb * CHUNK, db * F + (fb + 1) * CHUNK)
                    nc.tensor.matmul(
                        pn[:, fb : fb + 1],
                        sq[:, blk],
                        ones_sb[:],
                        start=(db == 0),
                        stop=(db == NDB - 1),
                    )
            cols = slice(bg * NCH, (bg + 1) * NCH)
            nc.vector.tensor_copy(qf_all[:, cols], pd[:])
            nc.vector.tensor_copy(m_all[:, cols], pn[:])

        def emit_A(b):
            q_bc = q_all[:, b * D3 : (b + 1) * D3]
            ftb = None
            if dma == "big" and mode != "comp":
                ftb = facts.tile([CHUNK, NCH * D3], fdt, tag="ftb")
                engs[b % len(engs)].dma_start(
                    ftb[:].rearrange("p (j d) -> p j d", j=NCH),
                    fcat.ap()[b].rearrange("(j p) d -> p j d", p=CHUNK),
                )
            for j in range(NCH):
                col = b * NCH + j
                if mode == "comp":
                    sl = ft0[:]
                elif dma == "big":
                    sl = ftb[:, j * D3 : (j + 1) * D3]
                else:
                    ft = facts.tile([CHUNK, D3], fdt, tag="ft")
                    r0 = slice(j * CHUNK, (j + 1) * CHUNK)
                    engs[col % len(engs)].dma_start(ft[:], fcat.ap()[b, r0, :])
                    sl = ft[:]
                if mode == "dma":
                    continue

                # Decoupled passes; dot -2*q.f on DVE (or Pool for a
                # dot_pool_k slice share), norm sum(f^2) on ACT/DVE/Pool.
                dot_eng = (
                    nc.gpsimd if (col % 16) < dot_pool_k else nc.vector
                )
                d_scr = scr.tile([CHUNK, D3], fdt, tag="dve_scr")
                dot_eng.scalar_tensor_tensor(
                    out=d_scr[:],
                    in0=sl,
                    scalar=-2.0,
                    in1=q_bc,
                    op0=mybir.AluOpType.mult,
                    op1=mybir.AluOpType.mult,
                    accum_out=qf_all[:, col : col + 1],
                )
                if (col % 8) < dve_k or (col % 16) < dve_k16:
                    d_scr2 = scr.tile([CHUNK, D3], fdt, tag="dve_scr")
                    nc.vector.scalar_tensor_tensor(
                        out=d_scr2[:],
                        in0=sl,
                        scalar=1.0,
                        in1=sl,
                        op0=mybir.AluOpType.mult,
                        op1=mybir.AluOpType.mult,
                        accum_out=m_all[:, col : col + 1],
                    )
                elif (col % 8) < dve_k + pool_k:
                    p_scr = scr.tile([CHUNK, D3], fdt, tag="pool_scr")
                    nc.gpsimd.scalar_tensor_tensor(
                        out=p_scr[:],
                        in0=sl,
                        scalar=1.0,
                        in1=sl,
                        op0=mybir.AluOpType.mult,
                        op1=mybir.AluOpType.mult,
                        accum_out=m_all[:, col : col + 1],
                    )
                else:
                    a_scr = scr.tile([CHUNK, D3], fdt, tag="act_scr")
                    nc.scalar.activation(
                        a_scr[:], sl, Sq, accum_out=m_all[:, col : col + 1]
                    )

        # Interleave A examples into the B stream so DVE/ACT A-work overlaps
        # the B DMA/PE pipeline instead of serializing at the end.
        order = []
        if NB:
            a_pos = {
                round((i + 1) * (BPC / (NA + 1))) for i in range(NA)
            } if NA else set()
            ai, bi2 = 0, 0
            for i in range(BPC):
                if i in a_pos and ai < NA:
                    order.append(("A", ai)); ai += 1
                elif bi2 < NB:
                    order.append(("B", bi2)); bi2 += 1
                else:
                    order.append(("A", ai)); ai += 1
        else:
            order = [("A", b) for b in range(NA)]

        loop_cm = tc.For_i(0, hw_repeat) if hw_repeat > 1 else None
        if loop_cm is not None:
            loop_cm.__enter__()
        for _rep in range(repeat):
          for kind, idx in order:
              if kind == "B":
                  emit_B(idx)
              else:
                  emit_A(idx)

        if loop_cm is not None:
            loop_cm.__exit__(None, None, None)

        if mode == "dma":
            nc.vector.tensor_copy(m_all[:, 0:NCOL], idx_sb[:, 0:NCOL])
            nc.vector.tensor_copy(qf_all[:, 0:NCOL], idx_sb[:, 0:NCOL])

        # Tail: scores = exp(-sq_dist), mask, per-example max over chunks.
        sc = small.tile([CHUNK, NCOL], _f32, tag="sc")
        nc.vector.tensor_add(m_all[:], m_all[:], qf_all[:])
        for b in range(BPC):
            bsl = slice(b * NCH, (b + 1) * NCH)
            nc.scalar.activation(
                sc[:, bsl],
                m_all[:, bsl],
                Ex,
                scale=-1.0,
                bias=qqn16[:, b : b + 1],
            )
        mask = small.tile([CHUNK, NCOL], _f32, tag="mask")
        nc.vector.tensor_tensor(
            mask[:], idx_sb[:], nb_rep[:], op=mybir.AluOpType.is_lt
        )
        msc = small.tile([CHUNK, NCOL], _f32, tag="msc")
        nc.vector.tensor_mul(msc[:], sc[:], mask[:])
        allmax = small.tile([CHUNK, BPC], _f32, tag="allmax")
        nc.vector.tensor_reduce(
            allmax[:],
            msc[:].rearrange("p (b j) -> p b j", b=BPC),
            axis=mybir.AxisListType.X,
            op=mybir.AluOpType.max,
        )
        nc.sync.dma_start(out_t.ap()[:, :], allmax[:])

    nc.compile()
    return nc


def _get_program():
    if "nc" not in _cache:
        _cache["nc"] = _build_program(**CFG)
    return _cache["nc"]


def _make_in_maps(rel, arg1, arg2, fact_rel, fact_arg1, fact_arg2, nb_facts,
                  dtype=None, nb_ex=None):
    _, ndt = _DT[dtype or CFG["dtype"]]
    if nb_ex is None:
        nb_ex = CFG.get("nb_ex", 0) if CFG.get("mode") == "hybridpe" else 0
    NB, NA = nb_ex, BPC - nb_ex
    NDB = D3 // CHUNK
    q_f32 = np.concatenate(
        [
            np.asarray(rel, dtype=np.float32),
            np.asarray(arg1, dtype=np.float32),
            np.asarray(arg2, dtype=np.float32),
        ],
        axis=1,
    )  # [B, 768]
    q_cat = q_f32.astype(ndt)
    nb_f32 = np.asarray(nb_facts).astype(np.float32)
    fact_cat = np.concatenate(
        [
            np.asarray(fact_rel, dtype=np.float32),
            np.asarray(fact_arg1, dtype=np.float32),
            np.asarray(fact_arg2, dtype=np.float32),
        ],
        axis=2,
    ).astype(ndt)  # [B, F, 768]

    in_maps = []
    for c in range(N_CORES):
        s = slice(c * BPC, (c + 1) * BPC)
        q_flat = q_cat[s].reshape(1, BPC * D3)
        nb_flat = np.repeat(nb_f32[s], NCH).reshape(1, NCOL)
        m = {
            "q_rep": np.ascontiguousarray(
                np.broadcast_to(q_flat, (CHUNK, BPC * D3))
            ),
            "fact_cat": fact_cat[s][:NA],
            "nb_rep": np.ascontiguousarray(np.broadcast_to(nb_flat, (CHUNK, NCOL))),
        }
        if NB:
            fb = fact_cat[s][NA:]  # [NB, F, 768]
            m["fact_t"] = np.ascontiguousarray(
                fb.transpose(0, 2, 1).reshape(NB, NDB, CHUNK, F)
            )
            qb = (-2.0 * q_f32[s][NA:]).astype(ndt)  # [NB, 768]
            m["q_t"] = np.ascontiguousarray(
                qb.reshape(NB, NDB, CHUNK).transpose(2, 0, 1).reshape(CHUNK, NB * NDB)
            )
        in_maps.append(m)
    return in_maps


def kernel(rel, arg1, arg2, fact_rel, fact_arg1, fact_arg2, nb_facts):
    nc = _get_program()
    in_maps = _make_in_maps(
        rel, arg1, arg2, fact_rel, fact_arg1, fact_arg2, nb_facts
    )
    res = run_bass_kernel_spmd(nc, in_maps, list(range(N_CORES))).results
    # res[c]["out"]: [128, BPC] per-partition chunk maxima; final 128-way max
    # per example happens here in the gather.
    out = np.concatenate(
        [np.asarray(res[c]["out"]).max(axis=0) for c in range(N_CORES)]
    )
    return out.astype(np.float32)
